# revision 1
# baseline (speedup 1.0000x reference)
"""Trainium2 Bass kernel for BaichuanAttention (hidden=5120, 40 heads, b=2, s=2048).

Tensor-parallel over heads across 8 NeuronCores: each core computes QKV for its
5 heads (sharded W_pack rows), flash-style causal attention, and a partial
o_proj (sharded W_o columns); partials are combined with an on-device
ReduceScatter and reassembled on the host.

Matmuls run as float32r (TF32-like fast fp32 path, ~1.5e-4 rel err).
"""

import math
import sys

for _p in ("/opt/trn_rl_repo",):
    if _p not in sys.path:
        sys.path.insert(0, _p)

import numpy as np

import concourse.bass as bass
import concourse.mybir as mybir
import concourse.tile as tile
from concourse import bacc, bass_utils

F32 = mybir.dt.float32
F32R = mybir.dt.float32r


class Cfg:
    def __init__(self, hidden=5120, n_heads=40, dh=128, B=2, S=2048, n_cores=8):
        self.hidden = hidden
        self.n_heads = n_heads
        self.dh = dh
        self.B = B
        self.S = S
        self.n_cores = n_cores
        assert dh == 128
        self.HL = n_heads // n_cores          # heads per core
        self.F = 3 * self.HL * dh             # per-core packed qkv rows
        self.FO = self.HL * dh                # per-core o_proj input width
        self.T = B * S                        # total tokens
        self.KT = hidden // 128               # contraction tiles for qkv
        self.TC = self.T // 512               # token chunks for qkv
        self.SQT = S // 128                   # q tiles per batch
        self.QC = S // 512                    # q chunks per batch
        self.OC = hidden // 512               # o chunks
        self.FTO = self.FO // 128             # attn feature tiles per core
        self.TG = max(1, self.T // 1024)      # reduce-scatter groups
        assert self.T % (self.TG * n_cores) == 0
        self.RS_ROWS = self.T // (self.TG * n_cores)  # out rows per core per group

    def key(self):
        return (self.hidden, self.n_heads, self.dh, self.B, self.S, self.n_cores)


def _ft_splits(n, cap=8):
    out = []
    while n > 0:
        take = min(cap, n)
        # avoid a tiny trailing pass
        if 0 < n - take < 3 and take > 4:
            take = n - 2
        out.append(take)
        n -= take
    return out


def build_program(cfg: Cfg, mode: str, phases: str = "ABC"):
    """mode: 'causal' (ignore mask input, causal skip), 'dense' (zero mask),
    'masked' (general additive mask input, pre-scaled by sqrt(dh) on host)."""
    assert mode in ("causal", "dense", "masked")
    c = cfg
    nc = bacc.Bacc("TRN2", target_bir_lowering=False, debug=False,
                   num_devices=c.n_cores)
    mask_ext = None
    xt = nc.dram_tensor("xt", [c.hidden, c.T], F32R, kind="ExternalInput").ap()
    wqkvt = nc.dram_tensor("wqkvt", [c.hidden, c.F], F32R,
                           kind="ExternalInput").ap()
    wot = nc.dram_tensor("wot", [c.FO, c.hidden], F32R,
                         kind="ExternalInput").ap()
    if mode == "masked":
        mask_ext = nc.dram_tensor("mask", [c.S, c.S], F32,
                                  kind="ExternalInput").ap()
    out_ext = nc.dram_tensor("out", [c.TG, c.RS_ROWS, c.hidden], F32,
                             kind="ExternalOutput").ap()

    inv_sqrt_dh = 1.0 / math.sqrt(c.dh)
    NEG = -1.0e9

    with tile.TileContext(nc) as tc:
        with tc.tile_pool(name="dram", bufs=1, space="DRAM") as dram:
            qkvt = dram.tile([c.F, c.T], F32R)
            partials = [dram.tile([c.T // c.TG, c.hidden], F32,
                                  tag=f"part{i}", name=f"part{i}")
                        for i in range(c.TG)]

            # ---------------- Phase A: QKV projection -------------------
            # qkvt[f, t] = sum_h wqkvt[h, f] * xt[h, t]
            do_a, do_b, do_c = ("A" in phases), ("B" in phases), ("C" in phases)
            wq_r = wqkvt.rearrange("(ko p) f -> p ko f", p=128)
            xt_r = xt.rearrange("(ko p) t -> p ko t", p=128)
            qkvt_r = qkvt.rearrange("(ft p) t -> ft p t", p=128)
            splits = _ft_splits(c.F // 128) if do_a else []
            with tc.tile_pool(name="qkv_w", bufs=1) as wpool, \
                 tc.tile_pool(name="qkv_x", bufs=6) as xpool, \
                 tc.tile_pool(name="qkv_o", bufs=8) as opool, \
                 tc.tile_pool(name="qkv_ps", bufs=8, space="PSUM") as pspool:
                ft0 = 0
                for nft in splits:
                    w_sb = wpool.tile([128, c.KT, nft * 128], F32R, tag="w")
                    for kq in range(c.KT):
                        nc.sync.dma_start(
                            w_sb[:, kq],
                            wq_r[:, kq, ft0 * 128:(ft0 + nft) * 128])
                    for tci in range(c.TC):
                        pss = [pspool.tile([128, 512], F32, tag="ps",
                                           name=f"ps{i}")
                               for i in range(nft)]
                        for k in range(c.KT):
                            x_sb = xpool.tile([128, 512], F32R, tag="x")
                            nc.sync.dma_start(
                                x_sb[:], xt_r[:, k, tci * 512:(tci + 1) * 512])
                            for i in range(nft):
                                nc.tensor.matmul(
                                    pss[i][:],
                                    w_sb[:, k, i * 128:(i + 1) * 128],
                                    x_sb[:],
                                    start=(k == 0), stop=(k == c.KT - 1))
                        for i in range(nft):
                            o_sb = opool.tile([128, 512], F32R, tag="o")
                            nc.vector.tensor_copy(o_sb[:], pss[i][:])
                            nc.sync.dma_start(
                                qkvt_r[ft0 + i, :, tci * 512:(tci + 1) * 512],
                                o_sb[:])
                    ft0 += nft

            # ---------------- Phase B: attention ------------------------
            with tc.tile_pool(name="att_at", bufs=1) as atpool:
              attnT = atpool.tile([128, c.FTO, c.T], F32R)
              with tc.tile_pool(name="att_const", bufs=1) as cpool, \
                 tc.tile_pool(name="att_in", bufs=2) as inpool, \
                 tc.tile_pool(name="att_v", bufs=1) as vpool, \
                 tc.tile_pool(name="att_p", bufs=5) as ppool, \
                 tc.tile_pool(name="att_pt", bufs=c.S // 128 + 2) as ptpool, \
                 tc.tile_pool(name="att_sm", bufs=2) as smpool, \
                 tc.tile_pool(name="att_ms", bufs=(4 if mode == "masked" else 1)) as mspool, \
                 tc.tile_pool(name="ps_s", bufs=3, space="PSUM") as ps_s, \
                 tc.tile_pool(name="ps_pt", bufs=2, space="PSUM") as ps_pt, \
                 tc.tile_pool(name="ps_at", bufs=2, space="PSUM") as ps_at, \
                 tc.tile_pool(name="ps_sm", bufs=1, space="PSUM") as ps_sm:

                ident = cpool.tile([128, 128], F32R)
                ones1 = cpool.tile([1, 128], F32R)
                with tc.tile_pool(name="att_tmp", bufs=1) as tmppool:
                    ident32 = tmppool.tile([128, 128], F32)
                    nc.gpsimd.memset(ident32[:], 0.0)
                    nc.gpsimd.affine_select(
                        out=ident32[:], in_=ident32[:],
                        compare_op=mybir.AluOpType.not_equal, fill=1.0,
                        base=0, pattern=[[-1, 128]], channel_multiplier=1)
                    nc.vector.tensor_copy(ident[:], ident32[:])
                    ones32 = tmppool.tile([1, 128], F32, tag="ones32")
                    nc.vector.memset(ones32[:], 1.0)
                    nc.vector.tensor_copy(ones1[:], ones32[:])
                cmasks = []
                if mode == "causal":
                    for off in range(4):
                        m = cpool.tile([128, 512], F32, tag=f"cm{off}",
                                       name=f"cm{off}")
                        nc.gpsimd.memset(m[:], 0.0)
                        # m[p, y] = 0 where y <= off*128 + p else NEG
                        nc.gpsimd.affine_select(
                            out=m[:], in_=m[:],
                            compare_op=mybir.AluOpType.is_ge, fill=NEG,
                            base=off * 128, pattern=[[-1, 512]],
                            channel_multiplier=1)
                        cmasks.append(m)

                for b in range(c.B if do_b else 0):
                    for h in range(c.HL):
                        q_sb = inpool.tile([128, c.S], F32R, tag="q")
                        k_sb = inpool.tile([128, c.S], F32R, tag="k")
                        v_sb = inpool.tile([128, c.S], F32R, tag="v")
                        t0 = b * c.S
                        nc.sync.dma_start(
                            q_sb[:], qkvt[h * 128:(h + 1) * 128, t0:t0 + c.S])
                        nc.sync.dma_start(
                            k_sb[:], qkvt[(c.HL + h) * 128:(c.HL + h + 1) * 128,
                                          t0:t0 + c.S])
                        nc.sync.dma_start(
                            v_sb[:], qkvt[(2 * c.HL + h) * 128:
                                          (2 * c.HL + h + 1) * 128,
                                          t0:t0 + c.S])
                        # V to token-major [128, st, dh]
                        v_tok = vpool.tile([128, c.SQT, 128], F32R)
                        for st in range(c.SQT):
                            vt_ps = ps_sm.tile([128, 128], F32R, tag="sm")
                            nc.tensor.matmul(vt_ps[:],
                                             v_sb[:, st * 128:(st + 1) * 128],
                                             ident[:], is_transpose=True)
                            nc.vector.tensor_copy(v_tok[:, st, :], vt_ps[:])

                        for qc in range(c.QC):
                            if mode == "causal":
                                nkt = 4 * (qc + 1)
                            else:
                                nkt = c.SQT
                            pts = [ptpool.tile([128, 512], F32R, tag="pt",
                                               name=f"pt{i}")
                                   for i in range(nkt)]
                            rqT_ps = ps_sm.tile([1, 512], F32R, tag="sm")
                            for qtl in range(4):
                                qt = qc * 4 + qtl
                                nkc = (qt // 4 + 1) if mode == "causal" \
                                    else c.S // 512
                                dsum = smpool.tile([128, 4], F32, tag="dsum")
                                for kc in range(nkc):
                                    s_ps = ps_s.tile([128, 512], F32, tag="s")
                                    nc.tensor.matmul(
                                        s_ps[:],
                                        q_sb[:, qt * 128:(qt + 1) * 128],
                                        k_sb[:, kc * 512:(kc + 1) * 512],
                                        start=True, stop=True)
                                    if mode == "causal" and kc == nkc - 1:
                                        nc.vector.tensor_tensor(
                                            s_ps[:], s_ps[:],
                                            cmasks[qt % 4][:],
                                            mybir.AluOpType.add)
                                    elif mode == "masked":
                                        m_sb = mspool.tile([128, 512], F32,
                                                           tag="m")
                                        nc.sync.dma_start(
                                            m_sb[:],
                                            mask_ext[qt * 128:(qt + 1) * 128,
                                                     kc * 512:(kc + 1) * 512])
                                        nc.vector.tensor_tensor(
                                            s_ps[:], s_ps[:], m_sb[:],
                                            mybir.AluOpType.add)
                                    p_sb = ppool.tile([128, 512], F32R,
                                                      tag="p")
                                    nc.scalar.activation(
                                        p_sb[:], s_ps[:],
                                        mybir.ActivationFunctionType.Exp,
                                        scale=inv_sqrt_dh,
                                        accum_out=dsum[:, kc:kc + 1])
                                    # transpose the four 128-blocks into pts
                                    for j in range(4):
                                        pt_ps = ps_pt.tile([128, 128], F32R,
                                                           tag="ptp")
                                        nc.tensor.matmul(
                                            pt_ps[:],
                                            p_sb[:, j * 128:(j + 1) * 128],
                                            ident[:], is_transpose=True)
                                        nc.vector.tensor_copy(
                                            pts[kc * 4 + j][:, qtl * 128:
                                                            (qtl + 1) * 128],
                                            pt_ps[:])
                                # 1/rowsum -> transposed into rqT_ps column
                                rqs = smpool.tile([128, 1], F32, tag="rqs")
                                nc.vector.tensor_reduce(
                                    rqs[:], dsum[:, :nkc],
                                    axis=mybir.AxisListType.X,
                                    op=mybir.AluOpType.add)
                                rq = smpool.tile([128, 1], F32, tag="rq")
                                nc.vector.reciprocal(rq[:], rqs[:])
                                rqr = smpool.tile([128, 1], F32R, tag="rqr")
                                nc.vector.tensor_copy(rqr[:], rq[:])
                                nc.tensor.matmul(
                                    rqT_ps[:, qtl * 128:(qtl + 1) * 128],
                                    rqr[:], ident[:], is_transpose=True)
                            rqT_sb = smpool.tile([1, 512], F32R, tag="rqT")
                            nc.vector.tensor_copy(rqT_sb[:], rqT_ps[:])
                            rqb_ps = ps_sm.tile([128, 512], F32, tag="sm")
                            nc.tensor.matmul(rqb_ps[:], ones1[:], rqT_sb[:],
                                             start=True, stop=True)
                            rqb_sb = smpool.tile([128, 512], F32, tag="rqb")
                            nc.vector.tensor_copy(rqb_sb[:], rqb_ps[:])
                            at_ps = ps_at.tile([128, 512], F32, tag="at")
                            for kt in range(nkt):
                                nc.tensor.matmul(
                                    at_ps[:], v_tok[:, kt, :], pts[kt][:],
                                    start=(kt == 0), stop=(kt == nkt - 1))
                            nc.vector.tensor_tensor(
                                attnT[:, h, t0 + qc * 512:t0 + (qc + 1) * 512],
                                at_ps[:], rqb_sb[:], mybir.AluOpType.mult)

              # ---------------- Phase C: o_proj + reduce-scatter ------
              wot_r = wot.rearrange("(ft p) o -> p ft o", p=128)
              with tc.tile_pool(name="op_w", bufs=3) as wopool, \
                   tc.tile_pool(name="op_o", bufs=6) as oopool, \
                   tc.tile_pool(name="op_ps", bufs=4, space="PSUM") as opps:
                  tt_per_g = c.T // c.TG // 128
                  for tg in range(c.TG if do_c else 0):
                      for oc in range(c.OC):
                          wo_sb = wopool.tile([128, c.FTO, 512], F32R,
                                              tag="wo")
                          nc.sync.dma_start(
                              wo_sb[:],
                              wot_r[:, :, oc * 512:(oc + 1) * 512])
                          for tl in range(tt_per_g):
                              tt = tg * tt_per_g + tl
                              ps = opps.tile([128, 512], F32, tag="ops")
                              for ft in range(c.FTO):
                                  nc.tensor.matmul(
                                      ps[:],
                                      attnT[:, ft, tt * 128:(tt + 1) * 128],
                                      wo_sb[:, ft, :],
                                      start=(ft == 0),
                                      stop=(ft == c.FTO - 1))
                              po_sb = oopool.tile([128, 512], F32, tag="po")
                              nc.vector.tensor_copy(po_sb[:], ps[:])
                              nc.sync.dma_start(
                                  partials[tg][tl * 128:(tl + 1) * 128,
                                               oc * 512:(oc + 1) * 512],
                                  po_sb[:])
                      rs_out = dram.tile([c.RS_ROWS, c.hidden], F32,
                                         tag="rs")
                      nc.gpsimd.collective_compute(
                          "ReduceScatter",
                          mybir.AluOpType.add,
                          replica_groups=[list(range(c.n_cores))],
                          ins=[partials[tg][:].opt()],
                          outs=[rs_out[:].opt()],
                      )
                      nc.gpsimd.dma_start(out_ext[tg], rs_out[:])

    nc.compile()
    return nc


# --------------------------------------------------------------------------
_CACHE = {}


def _get_program(cfg: Cfg, mode: str):
    key = (cfg.key(), mode)
    if key not in _CACHE:
        _CACHE[key] = build_program(cfg, mode)
    return _CACHE[key]


def prepare_inputs(cfg: Cfg, hidden_states, attention_mask, W_pack, W_o):
    """Host-side shard + layout prep. Returns (mode, in_maps)."""
    c = cfg
    X = np.asarray(hidden_states, dtype=np.float32).reshape(c.T, c.hidden)
    XT = np.ascontiguousarray(X.T)

    mask = np.asarray(attention_mask, dtype=np.float32).reshape(c.S, c.S)
    causal_ref = np.where(
        np.tril(np.ones((c.S, c.S), dtype=bool)), 0.0, -1e9
    ).astype(np.float32)
    if np.array_equal(mask, causal_ref):
        mode = "causal"
    elif not mask.any():
        mode = "dense"
    else:
        mode = "masked"

    W_pack = np.asarray(W_pack, dtype=np.float32)
    W_o = np.asarray(W_o, dtype=np.float32)
    H = c.hidden
    in_maps = []
    for g in range(c.n_cores):
        r0, r1 = g * c.FO, (g + 1) * c.FO
        wq = W_pack[r0:r1]
        wk = W_pack[H + r0:H + r1]
        wv = W_pack[2 * H + r0:2 * H + r1]
        wqkvT = np.ascontiguousarray(
            np.concatenate([wq, wk, wv], axis=0).T)       # [H, F]
        woT = np.ascontiguousarray(W_o[:, r0:r1].T)       # [FO, H]
        m = {"xt": XT, "wqkvt": wqkvT, "wot": woT}
        if mode == "masked":
            m["mask"] = np.ascontiguousarray(mask * math.sqrt(c.dh))
        in_maps.append(m)
    return mode, in_maps


def assemble_output(cfg: Cfg, results):
    c = cfg
    full = np.empty((c.T, c.hidden), dtype=np.float32)
    rows_g = c.T // c.TG
    for g in range(c.n_cores):
        o = results[g]["out"].reshape(c.TG, c.RS_ROWS, c.hidden)
        for tg in range(c.TG):
            a = tg * rows_g + g * c.RS_ROWS
            full[a:a + c.RS_ROWS] = o[tg]
    return full.reshape(c.B, c.S, c.hidden)


def kernel(hidden_states, attention_mask, W_pack, W_o):
    cfg = Cfg()
    mode, in_maps = prepare_inputs(cfg, hidden_states, attention_mask,
                                   W_pack, W_o)
    nc = _get_program(cfg, mode)
    res = bass_utils.run_bass_kernel_spmd(nc, in_maps,
                                          list(range(cfg.n_cores)))
    return assemble_output(cfg, res.results)



# revision 11
# speedup vs baseline: 1.6152x; 1.6152x over previous
"""Trainium2 Bass kernel for BaichuanAttention (hidden=5120, 40 heads, b=2, s=2048).

Tensor-parallel over heads across 8 NeuronCores, all matmuls in fp16
(full PE rate, fp32 PSUM accumulation):

  Phase A: per-core QKV projection (sharded W_pack rows), output
           feature-major qkvt in DRAM (fp16).
  Phase B: causal attention with transposed-scores formulation:
           S^T[k,q] = K_tile^T @ Q directly gives the P^T layout the PV
           matmul needs -- no per-tile transposes.  exp on the scalar
           engine (constant bias keeps P in fp16 range), row-sums via
           vector adds + gpsimd partition_all_reduce, V loaded
           token-major via DMA transpose.
  Phase C (fused into B's q-chunk loop): AllGather each finished
           attnT chunk across cores, then each core computes o_proj for
           its 640 hidden columns (full 5120-feature contraction) -- no
           ReduceScatter on the critical path; output is column-sharded.
"""

import math
import sys
from collections import deque

for _p in ("/opt/trn_rl_repo",):
    if _p not in sys.path:
        sys.path.insert(0, _p)

import numpy as np

import concourse.bass as bass
import concourse.bass_isa as bass_isa
import concourse.mybir as mybir
import concourse.tile as tile
from concourse import bacc, bass_utils

F16 = mybir.dt.float16
BF16 = mybir.dt.bfloat16
F32 = mybir.dt.float32


class Cfg:
    def __init__(self, hidden=5120, n_heads=40, dh=128, B=2, S=2048, n_cores=8):
        self.hidden = hidden
        self.n_heads = n_heads
        self.dh = dh
        self.B = B
        self.S = S
        self.n_cores = n_cores
        assert dh == 128
        self.HL = n_heads // n_cores          # heads per core (5)
        self.F = 3 * self.HL * dh             # per-core packed qkv rows (1920)
        self.FO = self.HL * dh                # per-core attn feature width (640)
        self.T = B * S                        # total tokens (4096)
        self.KT = hidden // 128               # contraction tiles for qkv (40)
        self.TC = self.T // 512               # token chunks for qkv (8)
        self.QC = S // 512                    # q chunks per batch (4)
        self.SQT = S // 128                   # k tiles per batch (16)
        self.JB = self.FO // 128              # out column blocks per core (5)

    def key(self):
        return (self.hidden, self.n_heads, self.dh, self.B, self.S, self.n_cores)


def build_program(cfg: Cfg, mode: str):
    """mode: 'causal' (causal skip + multiplicative tri masks),
    'dense' (no mask), 'masked' (additive mask input, pre-scaled and
    pre-transposed on host)."""
    assert mode in ("causal", "dense", "masked")
    c = cfg
    nc = bacc.Bacc("TRN2", target_bir_lowering=False, debug=False,
                   num_devices=c.n_cores)
    xt = nc.dram_tensor("xt", [c.hidden, c.T], F16, kind="ExternalInput").ap()
    wqkvt = nc.dram_tensor("wqkvt", [c.hidden, c.F], F16,
                           kind="ExternalInput").ap()
    wot = nc.dram_tensor("wot", [c.hidden, c.FO], F16,
                         kind="ExternalInput").ap()
    mask_ext = None
    if mode == "masked":
        # maskT[k, q] = mask[q, k] * sqrt(dh), fp32
        mask_ext = nc.dram_tensor("maskt", [c.S, c.S], F32,
                                  kind="ExternalInput").ap()
    # column-sharded transposed output: rows = this core's 640 hidden cols
    out_ext = nc.dram_tensor("out", [c.FO, c.T], F16,
                             kind="ExternalOutput").ap()

    inv_sqrt_dh = 1.0 / math.sqrt(c.dh)

    with tile.TileContext(nc) as tc:
        with tc.tile_pool(name="dram", bufs=1, space="DRAM") as dram:
            # q,k features in fp16; v features in bf16 (the softmax P tiles
            # must be bf16 for range, and the PV matmul needs matching dtypes)
            qkt = dram.tile([2 * c.FO, c.T], F16, tag="qkt", name="qkt")
            vt = dram.tile([c.FO, c.T], BF16, tag="vt", name="vt")
            stages = {}
            gaths = {}
            for qc in range(c.QC):
                for b in range(c.B):
                    stages[(qc, b)] = dram.tile(
                        [c.FO, 512], F16, tag=f"st{qc}_{b}", name=f"st{qc}_{b}")
                    gaths[(qc, b)] = dram.tile(
                        [c.n_cores, c.FO, 512], F16, tag=f"g{qc}_{b}",
                        name=f"g{qc}_{b}", addr_space="Shared")

            # ---------------- Phase A: QKV projection -------------------
            # qkvt[f, t] = sum_h wqkvt[h, f] * xt[h, t]
            wq_r = wqkvt.rearrange("(ko p) f -> p ko f", p=128)
            xt_r = xt.rearrange("(ko p) t -> p ko t", p=128)
            qkt_r = qkt.rearrange("(ft p) t -> ft p t", p=128)
            vt_r = vt.rearrange("(ft p) t -> ft p t", p=128)
            n_qk_ft = 2 * c.FO // 128  # 10
            splits = [8, 7]
            assert sum(splits) == c.F // 128
            with tc.tile_pool(name="qkv_w", bufs=2) as wpool, \
                 tc.tile_pool(name="qkv_x", bufs=6) as xpool, \
                 tc.tile_pool(name="qkv_o", bufs=8) as opool, \
                 tc.tile_pool(name="qkv_ps", bufs=8, space="PSUM") as pspool:
                ft0 = 0
                for nft in splits:
                    w_sb = wpool.tile([128, c.KT, nft * 128], F16, tag="w")
                    for kq in range(c.KT):
                        nc.sync.dma_start(
                            w_sb[:, kq],
                            wq_r[:, kq, ft0 * 128:(ft0 + nft) * 128])
                    for tci in range(c.TC):
                        pss = [pspool.tile([128, 512], F32, tag="ps",
                                           name=f"ps{i}")
                               for i in range(nft)]
                        for k in range(c.KT):
                            x_sb = xpool.tile([128, 512], F16, tag="x")
                            nc.sync.dma_start(
                                x_sb[:], xt_r[:, k, tci * 512:(tci + 1) * 512])
                            for i in range(nft):
                                nc.tensor.matmul(
                                    pss[i][:],
                                    w_sb[:, k, i * 128:(i + 1) * 128],
                                    x_sb[:],
                                    start=(k == 0), stop=(k == c.KT - 1))
                        for i in range(nft):
                            ft = ft0 + i
                            if ft < n_qk_ft:
                                o_sb = opool.tile([128, 512], F16, tag="o")
                                dst = qkt_r[ft, :, tci * 512:(tci + 1) * 512]
                            else:
                                o_sb = opool.tile([128, 512], BF16, tag="ov")
                                dst = vt_r[ft - n_qk_ft, :,
                                           tci * 512:(tci + 1) * 512]
                            nc.vector.tensor_copy(o_sb[:], pss[i][:])
                            nc.sync.dma_start(dst, o_sb[:])
                    ft0 += nft

            # ------------- Phase B + C: attention + o_proj --------------
            wot_r = wot.rearrange("(fb p) j -> p fb j", p=128)
            with tc.tile_pool(name="att_c", bufs=1) as cpool, \
                 tc.tile_pool(name="att_q", bufs=3) as qpool, \
                 tc.tile_pool(name="att_k", bufs=2) as kpool, \
                 tc.tile_pool(name="att_v", bufs=2) as vpool, \
                 tc.tile_pool(name="att_p", bufs=7) as ppool, \
                 tc.tile_pool(name="att_sm", bufs=2) as smpool, \
                 tc.tile_pool(name="att_o", bufs=3) as aopool, \
                 tc.tile_pool(name="att_ms", bufs=(4 if mode == "masked" else 1)) as mspool, \
                 tc.tile_pool(name="op_w", bufs=1) as wopool, \
                 tc.tile_pool(name="op_g", bufs=2) as gpool, \
                 tc.tile_pool(name="op_o", bufs=4) as copool, \
                 tc.tile_pool(name="ps_s", bufs=3, space="PSUM") as ps_s, \
                 tc.tile_pool(name="ps_pv", bufs=2, space="PSUM") as ps_pv, \
                 tc.tile_pool(name="ps_c", bufs=2, space="PSUM") as ps_c:

                # resident W_o slice: [128, 40, 640] fp16 (~51KB/partition)
                wo_sb = wopool.tile([128, c.KT, c.FO], F16)
                for fb in range(c.KT):
                    nc.sync.dma_start(wo_sb[:, fb], wot_r[:, fb, :])

                # multiplicative causal masks for the 4 diagonal k-tiles:
                # cm[j][p, y] = 1.0 where j*128 + p <= y else 0.0
                cmasks = []
                if mode == "causal":
                    with tc.tile_pool(name="att_tmp", bufs=1) as tmppool:
                        for j in range(4):
                            m32 = tmppool.tile([128, 512], F32, tag="m32",
                                               name=f"m32_{j}")
                            nc.gpsimd.memset(m32[:], 1.0)
                            nc.gpsimd.affine_select(
                                out=m32[:], in_=m32[:],
                                compare_op=mybir.AluOpType.is_ge, fill=0.0,
                                base=-j * 128, pattern=[[1, 512]],
                                channel_multiplier=-1)
                            m16 = cpool.tile([128, 512], BF16, tag=f"cm{j}",
                                             name=f"cm{j}")
                            nc.vector.tensor_copy(m16[:], m32[:])
                            cmasks.append(m16)

                def emit_attention(qc, b):
                    nk = 4 * (qc + 1) if mode == "causal" else c.SQT
                    t0 = b * c.S
                    q0 = qc * 512
                    SKEW = 3
                    stage_r = stages[(qc, b)].rearrange(
                        "(ft p) t -> ft p t", p=128)
                    for h in range(c.HL):
                        q_sb = qpool.tile([128, 512], F16, tag="q")
                        nc.sync.dma_start(
                            q_sb[:],
                            qkt[h * 128:(h + 1) * 128, t0 + q0:t0 + q0 + 512])
                        k_sb = kpool.tile([128, c.S], F16, tag="k")
                        nc.sync.dma_start(
                            k_sb[:, :nk * 128],
                            qkt[(c.HL + h) * 128:(c.HL + h + 1) * 128,
                                t0:t0 + nk * 128])
                        v_tok = vpool.tile([128, c.SQT, 128], BF16, tag="v")
                        nc.sync.dma_start(
                            v_tok[:, :nk, :],
                            vt[h * 128:(h + 1) * 128, t0:t0 + nk * 128],
                            transpose=True)
                        acc = smpool.tile([128, 512], F32, tag="acc")
                        accr = smpool.tile([128, 512], F32, tag="accr")
                        rq = smpool.tile([128, 512], F32, tag="rq")
                        pv_ps = ps_pv.tile([128, 512], F32, tag="pv")
                        p_tiles = {}
                        for kt in range(nk + SKEW):
                            if kt < nk:
                                s_ps = ps_s.tile([128, 512], F32, tag="s")
                                nc.tensor.matmul(
                                    s_ps[:],
                                    k_sb[:, kt * 128:(kt + 1) * 128],
                                    q_sb[:],
                                    start=True, stop=True)
                                if mode == "masked":
                                    m_sb = mspool.tile([128, 512], F32,
                                                       tag="m")
                                    nc.sync.dma_start(
                                        m_sb[:],
                                        mask_ext[kt * 128:(kt + 1) * 128,
                                                 q0:q0 + 512])
                                    nc.vector.tensor_tensor(
                                        s_ps[:], s_ps[:], m_sb[:],
                                        mybir.AluOpType.add)
                                p_sb = ppool.tile([128, 512], BF16, tag="p")
                                nc.scalar.activation(
                                    p_sb[:], s_ps[:],
                                    mybir.ActivationFunctionType.Exp,
                                    scale=inv_sqrt_dh)
                                if mode == "causal" and kt >= nk - 4:
                                    nc.vector.tensor_tensor(
                                        p_sb[:], p_sb[:],
                                        cmasks[kt - (nk - 4)][:],
                                        mybir.AluOpType.mult)
                                if kt == 0:
                                    nc.vector.tensor_copy(acc[:], p_sb[:])
                                else:
                                    nc.vector.tensor_tensor(
                                        acc[:], acc[:], p_sb[:],
                                        mybir.AluOpType.add)
                                p_tiles[kt] = p_sb
                            if kt >= SKEW:
                                j = kt - SKEW
                                nc.tensor.matmul(
                                    pv_ps[:], v_tok[:, j, :], p_tiles[j][:],
                                    start=(j == 0), stop=(j == nk - 1))
                                del p_tiles[j]
                        # denominators: all-reduce over partitions (k), then
                        # reciprocal; result broadcast on all partitions
                        nc.gpsimd.partition_all_reduce(
                            accr[:], acc[:], 128, bass_isa.ReduceOp.add)
                        nc.vector.reciprocal(rq[:], accr[:])
                        att_h = aopool.tile([128, 512], F16, tag="ao")
                        nc.vector.tensor_tensor(
                            att_h[:], pv_ps[:], rq[:], mybir.AluOpType.mult)
                        nc.sync.dma_start(stage_r[h], att_h[:])
                    nc.gpsimd.collective_compute(
                        "AllGather",
                        mybir.AluOpType.bypass,
                        replica_groups=[list(range(c.n_cores))],
                        ins=[stages[(qc, b)][:].opt()],
                        outs=[gaths[(qc, b)][:].opt()],
                    )

                def emit_oproj(qc, b):
                    # out[j, t] for this core's 640 hidden cols, 512 tokens
                    gath_r = gaths[(qc, b)].rearrange(
                        "g (ft p) t -> p (g ft) t", p=128)
                    g_sb = gpool.tile([128, c.n_cores * c.HL, 512], F16,
                                      tag="g")
                    nc.sync.dma_start(g_sb[:], gath_r)
                    tg0 = b * c.S + qc * 512
                    for jb in range(c.JB):
                        cps = ps_c.tile([128, 512], F32, tag="cps")
                        for f in range(c.KT):
                            nc.tensor.matmul(
                                cps[:],
                                wo_sb[:, f, jb * 128:(jb + 1) * 128],
                                g_sb[:, f, :],
                                start=(f == 0), stop=(f == c.KT - 1))
                        co = copool.tile([128, 512], F16, tag="co")
                        nc.vector.tensor_copy(co[:], cps[:])
                        nc.sync.dma_start(
                            out_ext[jb * 128:(jb + 1) * 128, tg0:tg0 + 512],
                            co[:])

                pending = deque()
                for qc in range(c.QC):
                    for b in range(c.B):
                        emit_attention(qc, b)
                        pending.append((qc, b))
                        if len(pending) > 2:
                            emit_oproj(*pending.popleft())
                while pending:
                    emit_oproj(*pending.popleft())

    nc.compile()
    return nc


# --------------------------------------------------------------------------
_CACHE = {}


def _get_program(cfg: Cfg, mode: str):
    key = (cfg.key(), mode)
    if key not in _CACHE:
        _CACHE[key] = build_program(cfg, mode)
    return _CACHE[key]


def prepare_inputs(cfg: Cfg, hidden_states, attention_mask, W_pack, W_o):
    """Host-side shard + layout prep. Returns (mode, in_maps)."""
    c = cfg
    X = np.asarray(hidden_states, dtype=np.float32).reshape(c.T, c.hidden)
    XT = np.ascontiguousarray(X.T.astype(np.float16))

    mask = np.asarray(attention_mask, dtype=np.float32).reshape(c.S, c.S)
    causal_ref = np.where(
        np.tril(np.ones((c.S, c.S), dtype=bool)), 0.0, -1e9
    ).astype(np.float32)
    if np.array_equal(mask, causal_ref):
        mode = "causal"
    elif not mask.any():
        mode = "dense"
    else:
        mode = "masked"

    W_pack = np.asarray(W_pack, dtype=np.float32)
    W_o = np.asarray(W_o, dtype=np.float32)
    H = c.hidden
    in_maps = []
    for g in range(c.n_cores):
        r0, r1 = g * c.FO, (g + 1) * c.FO
        wq = W_pack[r0:r1]
        wk = W_pack[H + r0:H + r1]
        wv = W_pack[2 * H + r0:2 * H + r1]
        wqkvT = np.ascontiguousarray(
            np.concatenate([wq, wk, wv], axis=0).T.astype(np.float16))
        woT = np.ascontiguousarray(W_o[r0:r1, :].T.astype(np.float16))
        m = {"xt": XT, "wqkvt": wqkvT, "wot": woT}
        if mode == "masked":
            m["maskt"] = np.ascontiguousarray((mask * math.sqrt(c.dh)).T)
        in_maps.append(m)
    return mode, in_maps


def assemble_output(cfg: Cfg, results):
    c = cfg
    full = np.empty((c.T, c.hidden), dtype=np.float32)
    for g in range(c.n_cores):
        o = results[g]["out"]  # [FO, T] fp16
        full[:, g * c.FO:(g + 1) * c.FO] = o.astype(np.float32).T
    return full.reshape(c.B, c.S, c.hidden)


def kernel(hidden_states, attention_mask, W_pack, W_o):
    cfg = Cfg()
    mode, in_maps = prepare_inputs(cfg, hidden_states, attention_mask,
                                   W_pack, W_o)
    nc = _get_program(cfg, mode)
    res = bass_utils.run_bass_kernel_spmd(nc, in_maps,
                                          list(range(cfg.n_cores)))
    return assemble_output(cfg, res.results)


# revision 15
# speedup vs baseline: 1.6362x; 1.0130x over previous
"""Trainium2 Bass kernel for BaichuanAttention (hidden=5120, 40 heads, b=2, s=2048).

Tensor-parallel over heads across 8 NeuronCores, all matmuls in fp16
(full PE rate, fp32 PSUM accumulation):

  Phase A: per-core QKV projection (sharded W_pack rows), output
           feature-major qkvt in DRAM (fp16).
  Phase B: causal attention with transposed-scores formulation:
           S^T[k,q] = K_tile^T @ Q directly gives the P^T layout the PV
           matmul needs -- no per-tile transposes.  exp on the scalar
           engine (constant bias keeps P in fp16 range), row-sums via
           vector adds + gpsimd partition_all_reduce, V loaded
           token-major via DMA transpose.
  Phase C (fused into B's q-chunk loop): AllGather each finished
           attnT chunk across cores, then each core computes o_proj for
           its 640 hidden columns (full 5120-feature contraction) -- no
           ReduceScatter on the critical path; output is column-sharded.
"""

import math
import sys
from collections import deque

for _p in ("/opt/trn_rl_repo",):
    if _p not in sys.path:
        sys.path.insert(0, _p)

import numpy as np

import concourse.bass as bass
import concourse.bass_isa as bass_isa
import concourse.mybir as mybir
import concourse.tile as tile
from concourse import bacc, bass_utils

F16 = mybir.dt.float16
BF16 = mybir.dt.bfloat16
F32 = mybir.dt.float32


class Cfg:
    def __init__(self, hidden=5120, n_heads=40, dh=128, B=2, S=2048, n_cores=8):
        self.hidden = hidden
        self.n_heads = n_heads
        self.dh = dh
        self.B = B
        self.S = S
        self.n_cores = n_cores
        assert dh == 128
        self.HL = n_heads // n_cores          # heads per core (5)
        self.F = 3 * self.HL * dh             # per-core packed qkv rows (1920)
        self.FO = self.HL * dh                # per-core attn feature width (640)
        self.T = B * S                        # total tokens (4096)
        self.KT = hidden // 128               # contraction tiles for qkv (40)
        self.TC = self.T // 512               # token chunks for qkv (8)
        self.QC = S // 512                    # q chunks per batch (4)
        self.SQT = S // 128                   # k tiles per batch (16)
        self.JB = self.FO // 128              # out column blocks per core (5)

    def key(self):
        return (self.hidden, self.n_heads, self.dh, self.B, self.S, self.n_cores)


def build_program(cfg: Cfg, mode: str):
    """mode: 'causal' (causal skip + multiplicative tri masks),
    'dense' (no mask), 'masked' (additive mask input, pre-scaled and
    pre-transposed on host)."""
    assert mode in ("causal", "dense", "masked")
    c = cfg
    nc = bacc.Bacc("TRN2", target_bir_lowering=False, debug=False,
                   num_devices=c.n_cores)
    xt = nc.dram_tensor("xt", [c.hidden, c.T], F16, kind="ExternalInput").ap()
    wqkvt = nc.dram_tensor("wqkvt", [c.hidden, c.F], F16,
                           kind="ExternalInput").ap()
    wot = nc.dram_tensor("wot", [c.hidden, c.FO], F16,
                         kind="ExternalInput").ap()
    mask_ext = None
    if mode == "masked":
        # maskT[k, q] = mask[q, k] * sqrt(dh), fp32
        mask_ext = nc.dram_tensor("maskt", [c.S, c.S], F32,
                                  kind="ExternalInput").ap()
    # column-sharded transposed output: rows = this core's 640 hidden cols
    out_ext = nc.dram_tensor("out", [c.FO, c.T], F16,
                             kind="ExternalOutput").ap()

    inv_sqrt_dh = 1.0 / math.sqrt(c.dh)

    with tile.TileContext(nc) as tc:
        with tc.tile_pool(name="dram", bufs=1, space="DRAM") as dram:
            # q,k features in fp16; v features in bf16 (the softmax P tiles
            # must be bf16 for range, and the PV matmul needs matching dtypes)
            qkt = dram.tile([2 * c.FO, c.T], F16, tag="qkt", name="qkt")
            vt = dram.tile([c.FO, c.T], BF16, tag="vt", name="vt")
            stages = {}
            gaths = {}
            for qc in range(c.QC):
                for b in range(c.B):
                    stages[(qc, b)] = dram.tile(
                        [c.FO, 512], F16, tag=f"st{qc}_{b}", name=f"st{qc}_{b}")
                    gaths[(qc, b)] = dram.tile(
                        [c.n_cores, c.FO, 512], F16, tag=f"g{qc}_{b}",
                        name=f"g{qc}_{b}", addr_space="Shared")

            # ---------------- Phase A: QKV projection -------------------
            # qkvt[f, t] = sum_h wqkvt[h, f] * xt[h, t]
            wq_r = wqkvt.rearrange("(ko p) f -> p ko f", p=128)
            xt_r = xt.rearrange("(ko p) t -> p ko t", p=128)
            qkt_r = qkt.rearrange("(ft p) t -> ft p t", p=128)
            vt_r = vt.rearrange("(ft p) t -> ft p t", p=128)
            n_qk_ft = 2 * c.FO // 128  # 10
            splits = [8, 7]
            assert sum(splits) == c.F // 128
            with tc.tile_pool(name="qkv_w", bufs=2) as wpool, \
                 tc.tile_pool(name="qkv_x", bufs=6) as xpool, \
                 tc.tile_pool(name="qkv_o", bufs=8) as opool, \
                 tc.tile_pool(name="qkv_ps", bufs=8, space="PSUM") as pspool:
                ft0 = 0
                for nft in splits:
                    w_sb = wpool.tile([128, c.KT, nft * 128], F16, tag="w")
                    for kq in range(c.KT):
                        nc.sync.dma_start(
                            w_sb[:, kq],
                            wq_r[:, kq, ft0 * 128:(ft0 + nft) * 128])
                    for tci in range(c.TC):
                        pss = [pspool.tile([128, 512], F32, tag="ps",
                                           name=f"ps{i}")
                               for i in range(nft)]
                        for k in range(c.KT):
                            x_sb = xpool.tile([128, 512], F16, tag="x")
                            # scalar-engine HWDGE queue: x loads don't queue
                            # behind the 40 weight DMAs on the sync queue
                            nc.scalar.dma_start(
                                x_sb[:], xt_r[:, k, tci * 512:(tci + 1) * 512])
                            for i in range(nft):
                                nc.tensor.matmul(
                                    pss[i][:],
                                    w_sb[:, k, i * 128:(i + 1) * 128],
                                    x_sb[:],
                                    start=(k == 0), stop=(k == c.KT - 1))
                        for i in range(nft):
                            ft = ft0 + i
                            if ft < n_qk_ft:
                                o_sb = opool.tile([128, 512], F16, tag="o")
                                dst = qkt_r[ft, :, tci * 512:(tci + 1) * 512]
                            else:
                                o_sb = opool.tile([128, 512], BF16, tag="ov")
                                dst = vt_r[ft - n_qk_ft, :,
                                           tci * 512:(tci + 1) * 512]
                            nc.vector.tensor_copy(o_sb[:], pss[i][:])
                            nc.sync.dma_start(dst, o_sb[:])
                    ft0 += nft

            # ------------- Phase B + C: attention + o_proj --------------
            wot_r = wot.rearrange("(fb p) j -> p fb j", p=128)
            with tc.tile_pool(name="att_c", bufs=1) as cpool, \
                 tc.tile_pool(name="att_q", bufs=3) as qpool, \
                 tc.tile_pool(name="att_k", bufs=2) as kpool, \
                 tc.tile_pool(name="att_v", bufs=2) as vpool, \
                 tc.tile_pool(name="att_p", bufs=7) as ppool, \
                 tc.tile_pool(name="att_sm", bufs=2) as smpool, \
                 tc.tile_pool(name="att_o", bufs=3) as aopool, \
                 tc.tile_pool(name="att_ms", bufs=(4 if mode == "masked" else 1)) as mspool, \
                 tc.tile_pool(name="op_w", bufs=1) as wopool, \
                 tc.tile_pool(name="op_g", bufs=2) as gpool, \
                 tc.tile_pool(name="op_o", bufs=4) as copool, \
                 tc.tile_pool(name="ps_s", bufs=3, space="PSUM") as ps_s, \
                 tc.tile_pool(name="ps_pv", bufs=2, space="PSUM") as ps_pv, \
                 tc.tile_pool(name="ps_c", bufs=2, space="PSUM") as ps_c:

                # resident W_o slice: [128, 40, 640] fp16 (~51KB/partition)
                wo_sb = wopool.tile([128, c.KT, c.FO], F16)
                for fb in range(c.KT):
                    nc.sync.dma_start(wo_sb[:, fb], wot_r[:, fb, :])

                # multiplicative causal masks for the 4 diagonal k-tiles:
                # cm[j][p, y] = 1.0 where j*128 + p <= y else 0.0
                cmasks = []
                if mode == "causal":
                    with tc.tile_pool(name="att_tmp", bufs=1) as tmppool:
                        for j in range(4):
                            m32 = tmppool.tile([128, 512], F32, tag="m32",
                                               name=f"m32_{j}")
                            nc.gpsimd.memset(m32[:], 1.0)
                            nc.gpsimd.affine_select(
                                out=m32[:], in_=m32[:],
                                compare_op=mybir.AluOpType.is_ge, fill=0.0,
                                base=-j * 128, pattern=[[1, 512]],
                                channel_multiplier=-1)
                            m16 = cpool.tile([128, 512], BF16, tag=f"cm{j}",
                                             name=f"cm{j}")
                            nc.vector.tensor_copy(m16[:], m32[:])
                            cmasks.append(m16)

                def emit_attention(qc, b):
                    nk = 4 * (qc + 1) if mode == "causal" else c.SQT
                    t0 = b * c.S
                    q0 = qc * 512
                    SKEW = 3
                    stage_r = stages[(qc, b)].rearrange(
                        "(ft p) t -> ft p t", p=128)

                    def head_tail(h, acc, pv_ps):
                        # denominators: all-reduce over partitions (k), then
                        # reciprocal; result broadcast on all partitions
                        accr = smpool.tile([128, 512], F32, tag="accr")
                        rq = smpool.tile([128, 512], F32, tag="rq")
                        nc.gpsimd.partition_all_reduce(
                            accr[:], acc[:], 128, bass_isa.ReduceOp.add)
                        nc.vector.reciprocal_approx_fast(rq[:], accr[:])
                        att_h = aopool.tile([128, 512], F16, tag="ao")
                        nc.vector.tensor_tensor(
                            att_h[:], pv_ps[:], rq[:], mybir.AluOpType.mult)
                        nc.sync.dma_start(stage_r[h], att_h[:])

                    tails = []
                    for h in range(c.HL):
                        q_sb = qpool.tile([128, 512], F16, tag="q")
                        nc.sync.dma_start(
                            q_sb[:],
                            qkt[h * 128:(h + 1) * 128, t0 + q0:t0 + q0 + 512])
                        k_sb = kpool.tile([128, c.S], F16, tag="k")
                        nc.sync.dma_start(
                            k_sb[:, :nk * 128],
                            qkt[(c.HL + h) * 128:(c.HL + h + 1) * 128,
                                t0:t0 + nk * 128])
                        v_tok = vpool.tile([128, c.SQT, 128], BF16, tag="v")
                        nc.sync.dma_start(
                            v_tok[:, :nk, :],
                            vt[h * 128:(h + 1) * 128, t0:t0 + nk * 128],
                            transpose=True)
                        acc = smpool.tile([128, 512], F32, tag="acc")
                        pv_ps = ps_pv.tile([128, 512], F32, tag="pv")
                        p_tiles = {}
                        for kt in range(nk + SKEW):
                            if kt < nk:
                                s_ps = ps_s.tile([128, 512], F32, tag="s")
                                nc.tensor.matmul(
                                    s_ps[:],
                                    k_sb[:, kt * 128:(kt + 1) * 128],
                                    q_sb[:],
                                    start=True, stop=True)
                                if mode == "masked":
                                    m_sb = mspool.tile([128, 512], F32,
                                                       tag="m")
                                    nc.sync.dma_start(
                                        m_sb[:],
                                        mask_ext[kt * 128:(kt + 1) * 128,
                                                 q0:q0 + 512])
                                    nc.vector.tensor_tensor(
                                        s_ps[:], s_ps[:], m_sb[:],
                                        mybir.AluOpType.add)
                                p_sb = ppool.tile([128, 512], BF16, tag="p")
                                nc.scalar.activation(
                                    p_sb[:], s_ps[:],
                                    mybir.ActivationFunctionType.Exp,
                                    scale=inv_sqrt_dh)
                                if mode == "causal" and kt >= nk - 4:
                                    nc.vector.tensor_tensor(
                                        p_sb[:], p_sb[:],
                                        cmasks[kt - (nk - 4)][:],
                                        mybir.AluOpType.mult)
                                if kt == 0:
                                    nc.vector.tensor_copy(acc[:], p_sb[:])
                                else:
                                    nc.vector.tensor_tensor(
                                        acc[:], acc[:], p_sb[:],
                                        mybir.AluOpType.add)
                                p_tiles[kt] = p_sb
                            if kt >= SKEW:
                                j = kt - SKEW
                                nc.tensor.matmul(
                                    pv_ps[:], v_tok[:, j, :], p_tiles[j][:],
                                    start=(j == 0), stop=(j == nk - 1))
                                del p_tiles[j]
                        # skew the normalize tail by one head so the gpsimd
                        # all-reduce latency hides under the next head's work
                        if tails:
                            head_tail(*tails.pop())
                        tails.append((h, acc, pv_ps))
                    while tails:
                        head_tail(*tails.pop())
                    nc.gpsimd.collective_compute(
                        "AllGather",
                        mybir.AluOpType.bypass,
                        replica_groups=[list(range(c.n_cores))],
                        ins=[stages[(qc, b)][:].opt()],
                        outs=[gaths[(qc, b)][:].opt()],
                    )

                def emit_oproj(qc, b):
                    # out[j, t] for this core's 640 hidden cols, 512 tokens
                    gath_r = gaths[(qc, b)].rearrange(
                        "g (ft p) t -> p (g ft) t", p=128)
                    g_sb = gpool.tile([128, c.n_cores * c.HL, 512], F16,
                                      tag="g")
                    nc.sync.dma_start(g_sb[:], gath_r)
                    tg0 = b * c.S + qc * 512
                    for jb in range(c.JB):
                        cps = ps_c.tile([128, 512], F32, tag="cps")
                        for f in range(c.KT):
                            nc.tensor.matmul(
                                cps[:],
                                wo_sb[:, f, jb * 128:(jb + 1) * 128],
                                g_sb[:, f, :],
                                start=(f == 0), stop=(f == c.KT - 1))
                        co = copool.tile([128, 512], F16, tag="co")
                        nc.vector.tensor_copy(co[:], cps[:])
                        nc.sync.dma_start(
                            out_ext[jb * 128:(jb + 1) * 128, tg0:tg0 + 512],
                            co[:])

                pending = deque()
                for qc in range(c.QC):
                    for b in range(c.B):
                        emit_attention(qc, b)
                        pending.append((qc, b))
                        if len(pending) > 2:
                            emit_oproj(*pending.popleft())
                while pending:
                    emit_oproj(*pending.popleft())

    nc.compile()
    return nc


# --------------------------------------------------------------------------
_CACHE = {}


def _get_program(cfg: Cfg, mode: str):
    key = (cfg.key(), mode)
    if key not in _CACHE:
        _CACHE[key] = build_program(cfg, mode)
    return _CACHE[key]


def prepare_inputs(cfg: Cfg, hidden_states, attention_mask, W_pack, W_o):
    """Host-side shard + layout prep. Returns (mode, in_maps)."""
    c = cfg
    X = np.asarray(hidden_states, dtype=np.float32).reshape(c.T, c.hidden)
    XT = np.ascontiguousarray(X.T.astype(np.float16))

    mask = np.asarray(attention_mask, dtype=np.float32).reshape(c.S, c.S)
    causal_ref = np.where(
        np.tril(np.ones((c.S, c.S), dtype=bool)), 0.0, -1e9
    ).astype(np.float32)
    if np.array_equal(mask, causal_ref):
        mode = "causal"
    elif not mask.any():
        mode = "dense"
    else:
        mode = "masked"

    W_pack = np.asarray(W_pack, dtype=np.float32)
    W_o = np.asarray(W_o, dtype=np.float32)
    H = c.hidden
    in_maps = []
    for g in range(c.n_cores):
        r0, r1 = g * c.FO, (g + 1) * c.FO
        wq = W_pack[r0:r1]
        wk = W_pack[H + r0:H + r1]
        wv = W_pack[2 * H + r0:2 * H + r1]
        wqkvT = np.ascontiguousarray(
            np.concatenate([wq, wk, wv], axis=0).T.astype(np.float16))
        woT = np.ascontiguousarray(W_o[r0:r1, :].T.astype(np.float16))
        m = {"xt": XT, "wqkvt": wqkvT, "wot": woT}
        if mode == "masked":
            m["maskt"] = np.ascontiguousarray((mask * math.sqrt(c.dh)).T)
        in_maps.append(m)
    return mode, in_maps


def assemble_output(cfg: Cfg, results):
    c = cfg
    full = np.empty((c.T, c.hidden), dtype=np.float32)
    for g in range(c.n_cores):
        o = results[g]["out"]  # [FO, T] fp16
        full[:, g * c.FO:(g + 1) * c.FO] = o.astype(np.float32).T
    return full.reshape(c.B, c.S, c.hidden)


def kernel(hidden_states, attention_mask, W_pack, W_o):
    cfg = Cfg()
    mode, in_maps = prepare_inputs(cfg, hidden_states, attention_mask,
                                   W_pack, W_o)
    nc = _get_program(cfg, mode)
    res = bass_utils.run_bass_kernel_spmd(nc, in_maps,
                                          list(range(cfg.n_cores)))
    return assemble_output(cfg, res.results)
